# revision 5
# baseline (speedup 1.0000x reference)
"""GroupedEmbedding lookup on 8 Trainium2 NeuronCores.

Sharding: table-wise (torchrec-style), 2 tables per core. Each core
holds its own [2*R, D] weight slab and that slab's index slices; its
output is a contiguous [2*L, D] block of the final [T*L, D] output, so
the un-shard is a plain concatenation (no device all-to-all needed).

Device algorithm (per core), v2 "MoE-style" path:
  One dma_gather + one dma_scatter_add extended instruction per
  (output-segment, weight-block) bucket. The int16 index limit of these
  instructions forces two-level blocking:
   - output positions are processed in NSEG=8 segments of SEGSZ=32768
     consecutive lookups (scatter offsets are segment-relative int16);
     each segment's lookups belong to exactly one table (SEGSZ | L);
   - each table's rows are split into BPT=7 blocks of BLKSZ=32768 rows
     (gather offsets are block-relative int16).
  Host pre-buckets each segment's lookups by weight block (stable
  counting sort). Per bucket: dma_gather pulls the bucket's 256B rows
  into SBUF (token i -> partition i%128, col i//128), dma_scatter_add
  writes them to their original positions in the output segment (the
  PJRT path zero-initializes outputs via donated buffers, each position
  is written exactly once, so += is =). Bucket counts are runtime
  register loads, so one compiled program serves any input data; idx
  buffers are padded to NPAD with trailing -1 (the documented "ignored"
  terminator) and replicated across the 8 gpsimd-core 16-partition
  windows as the ucode requires. single_packet=False is required:
  single-packet mode breaks above 1024 descriptors (64-descriptor HW
  packet ceiling x 16 SDMA engines).

  Two host-side orderings matter: buckets are sorted by ascending row
  offset so each gather's 256B HBM reads sweep its 8MB block nearly
  sequentially (~1.5KB mean stride; measured 29% faster than random
  order), and bucket counts ride in registers so the compiled program is
  input-independent. Measured per-iteration device time (8 cores, slope
  timing): ~2.50ms vs ~3.15ms for the indirect-DMA fallback.

Fallback path (bucket overflow/empty, i.e. severely non-uniform index
distributions): the original indirect_dma_start kernel, 128 rows per
instruction — slower but fully general.
"""
from contextlib import ExitStack
from dataclasses import dataclass
from functools import cached_property

import numpy as np

import concourse.bass as bass
import concourse.bacc as bacc
import concourse.mybir as mybir
from concourse.bass_utils import run_bass_kernel_spmd

T = 16          # tables
R = 200000      # rows per table
D = 64          # embedding dim (64 f32 = 256B rows)
L = 131072      # lookups per table
NCORES = 8
TPC = T // NCORES
NROWS = TPC * R     # 400000 rows per core
N = TPC * L         # 262144 lookups per core


# ---------------------------------------------------------------- v2 path

@dataclass(frozen=True)
class Cfg:
    SEGSZ: int = 32768   # output segment (int16 position limit)
    BLKSZ: int = 32768   # weight block (int16 row-offset limit)
    NPAD: int = 5760     # bucket capacity, mult of 128 (mean 5369 + 5.8s)
    NSLOT: int = 6       # data-tile pipeline slots
    LOOKAHEAD: int = 3   # gathers in flight before first scatter
    NQ: int = 2          # SWDGE queues
    HALF_OUT: bool = False  # cast to bf16 on-chip, 128B scatter descriptors

    @cached_property
    def SPT(self):
        return L // self.SEGSZ

    @cached_property
    def NSEG(self):
        return TPC * self.SPT

    @cached_property
    def BPT(self):
        return -(-R // self.BLKSZ)

    @cached_property
    def IDXCOLS(self):
        return self.NPAD // 16


CFG = Cfg()
_NC_CACHE = {}


def build_nc(cfg: Cfg = CFG, repeat=1):
    if (cfg, repeat) in _NC_CACHE:
        return _NC_CACHE[(cfg, repeat)]
    NSEG, BPT, IDXCOLS = cfg.NSEG, cfg.BPT, cfg.IDXCOLS
    NPAD, NSLOT, LOOKAHEAD = cfg.NPAD, cfg.NSLOT, cfg.LOOKAHEAD
    SEGSZ, BLKSZ, SPT = cfg.SEGSZ, cfg.BLKSZ, cfg.SPT
    NREG = LOOKAHEAD + 2
    assert NPAD % 128 == 0 and L % SEGSZ == 0 and NSEG % 2 == 0
    assert NSLOT >= LOOKAHEAD + 2

    # Bacc: its compile() handles the DMAGatherAnt/ScatterAddAnt extended
    # instructions (insert_library_loads + InstISA codegen); raw Bass +
    # walrus codegen rejects them ("ISA wrong length").
    nc = bacc.Bacc("TRN2", debug=False, num_swdge_queues=cfg.NQ)

    def gq(k):
        return (0, 0, 0, k % 2 * 2)[cfg.NQ - 1]

    def sq(k):
        return (0, 1, 1, k % 2 * 2 + 1)[cfg.NQ - 1]

    w = nc.dram_tensor("w", [NROWS, D], mybir.dt.float32,
                       kind="ExternalInput")
    gidx = nc.dram_tensor("gidx", [NSEG * 128, BPT * IDXCOLS],
                          mybir.dt.int16, kind="ExternalInput")
    sidx = nc.dram_tensor("sidx", [NSEG * 128, BPT * IDXCOLS],
                          mybir.dt.int16, kind="ExternalInput")
    cnt = nc.dram_tensor("cnt", [1, NSEG * BPT], mybir.dt.int32,
                         kind="ExternalInput")
    if cfg.HALF_OUT:
        # bf16 rows padded to 256B (scatter stride must divide by 256B)
        out = nc.dram_tensor("out", [N, 2 * D], mybir.dt.bfloat16,
                             kind="ExternalOutput")
    else:
        out = nc.dram_tensor("out", [N, D], mybir.dt.float32,
                             kind="ExternalOutput")

    with ExitStack() as ctx:
        gtile = [ctx.enter_context(
            nc.sbuf_tensor(f"gtile{p}", [128, BPT * IDXCOLS], mybir.dt.int16))
            for p in range(2)]
        stile = [ctx.enter_context(
            nc.sbuf_tensor(f"stile{p}", [128, BPT * IDXCOLS], mybir.dt.int16))
            for p in range(2)]
        ctile = ctx.enter_context(
            nc.sbuf_tensor("ctile", [1, NSEG * BPT], mybir.dt.int32))
        data = [ctx.enter_context(
            nc.sbuf_tensor(f"data{i}", [128, (NPAD // 128) * D],
                           mybir.dt.float32))
            for i in range(NSLOT)]
        data3 = [d.ap().rearrange("p (a b) -> p a b", b=D) for d in data]
        if cfg.HALF_OUT:
            half = [ctx.enter_context(
                nc.sbuf_tensor(f"half{i}", [128, (NPAD // 128) * D],
                               mybir.dt.bfloat16))
                for i in range(NSLOT)]
            half3 = [h.ap().rearrange("p (a b) -> p a b", b=D) for h in half]
            c_sems = [ctx.enter_context(nc.semaphore(f"c_sem{i}"))
                      for i in range(NSLOT)]

        cnt_sem = ctx.enter_context(nc.semaphore("cnt_sem"))
        idx_sems = [ctx.enter_context(nc.semaphore(f"idx_sem{p}"))
                    for p in range(2)]
        g_sems = [ctx.enter_context(nc.semaphore(f"g_sem{i}"))
                  for i in range(NSLOT)]
        s_sems = [ctx.enter_context(nc.semaphore(f"s_sem{i}"))
                  for i in range(NSLOT)]
        block = ctx.enter_context(nc.Block())

        @block.sync
        def _(sync):
            sync.dma_start(ctile[:], cnt.ap()).then_inc(cnt_sem, 16)
            for gs in range(NSEG * repeat):
                s = gs % NSEG
                p = gs % 2
                if gs >= 2:
                    # idx tiles of parity p were last read (at desc-gen
                    # time) by segment gs-2; wait for each of its bucket
                    # DMAs (completion implies the desc-gen read is done).
                    for kk in range((gs - 2) * BPT, (gs - 1) * BPT):
                        sync.wait_ge(g_sems[kk % NSLOT],
                                     16 * (kk // NSLOT + 1))
                        sync.wait_ge(s_sems[kk % NSLOT],
                                     16 * (kk // NSLOT + 1))
                sync.dma_start(gtile[p][:], gidx.ap()[s * 128:(s + 1) * 128, :]
                               ).then_inc(idx_sems[p], 16)
                sync.dma_start(stile[p][:], sidx.ap()[s * 128:(s + 1) * 128, :]
                               ).then_inc(idx_sems[p], 16)

        if cfg.HALF_OUT:
            # fp32 -> bf16 cast between gather and scatter, split across
            # the DVE and ACT engines (both otherwise idle).
            def cast_block(eng, parity):
                for k in range(NSEG * repeat * BPT):
                    if k % 2 != parity:
                        continue
                    slot, rnd = k % NSLOT, k // NSLOT
                    eng.wait_ge(g_sems[slot], 16 * (rnd + 1))
                    if rnd > 0:
                        eng.wait_ge(s_sems[slot], 16 * rnd)
                    if parity == 0:
                        inst = eng.tensor_copy(half[slot][:], data[slot][:])
                    else:
                        inst = eng.copy(half[slot][:], data[slot][:])
                    inst.then_inc(c_sems[slot], 1)

            @block.vector
            def _(vector):
                cast_block(vector, 0)

            @block.scalar
            def _(scalar):
                cast_block(scalar, 1)

        @block.gpsimd
        def _(gpsimd):
            regs = [gpsimd.alloc_register(f"cnt_reg{i}") for i in range(NREG)]
            gpsimd.wait_ge(cnt_sem, 16)
            pend = []

            def issue_scatter(k):
                s = (k // BPT) % NSEG
                b = k % BPT
                slot, rnd = k % NSLOT, k // NSLOT
                sidx_ap = stile[(k // BPT) % 2][:,
                                                b * IDXCOLS:(b + 1) * IDXCOLS]
                if cfg.HALF_OUT:
                    gpsimd.wait_ge(c_sems[slot], rnd + 1)
                    gpsimd.dma_scatter_add(
                        out.ap()[s * SEGSZ:(s + 1) * SEGSZ, 0:D],
                        half3[slot], sidx_ap,
                        NPAD, regs[k % NREG], D, elem_step=2 * D,
                        queue_num=sq(k), single_packet=False,
                    ).then_inc(s_sems[slot], 16)
                    return
                gpsimd.wait_ge(g_sems[slot], 16 * (rnd + 1))
                gpsimd.dma_scatter_add(
                    out.ap()[s * SEGSZ:(s + 1) * SEGSZ, :],
                    data3[slot], sidx_ap,
                    NPAD, regs[k % NREG], D, elem_step=D,
                    queue_num=sq(k), single_packet=False,
                ).then_inc(s_sems[slot], 16)

            for gs in range(NSEG * repeat):
                s = gs % NSEG
                t = s // SPT
                gpsimd.wait_ge(idx_sems[gs % 2], 32 * (gs // 2 + 1))
                for b in range(BPT):
                    k = gs * BPT + b
                    slot, rnd = k % NSLOT, k // NSLOT
                    if rnd > 0:
                        gpsimd.wait_ge(s_sems[slot], 16 * rnd)
                    gpsimd.reg_load(regs[k % NREG],
                                    ctile[:, (s * BPT + b):(s * BPT + b) + 1])
                    r0 = t * R + b * BLKSZ
                    r1 = t * R + min(R, (b + 1) * BLKSZ)
                    gpsimd.dma_gather(
                        data3[slot],
                        w.ap()[r0:r1, :],
                        gtile[gs % 2][:, b * IDXCOLS:(b + 1) * IDXCOLS],
                        NPAD, regs[k % NREG], D, elem_step=D,
                        queue_num=gq(k), single_packet=False,
                    ).then_inc(g_sems[slot], 16)
                    pend.append(k)
                    if len(pend) > LOOKAHEAD:
                        issue_scatter(pend.pop(0))
            for k in pend:
                issue_scatter(k)

    nc.compile()
    _NC_CACHE[(cfg, repeat)] = nc
    return nc


def shard_inputs(indices: np.ndarray, weights: np.ndarray, cfg: Cfg = CFG):
    """Full inputs -> per-core in_maps (bucketed, wrapped, replicated).

    Returns None when a bucket overflows NPAD or a feasible bucket is
    empty (won't happen for uniform indices) -> caller falls back to the
    generic v1 path.
    """
    NSEG, BPT, IDXCOLS = cfg.NSEG, cfg.BPT, cfg.IDXCOLS
    in_maps = []
    for c in range(NCORES):
        t0 = c * TPC
        w_c = np.ascontiguousarray(weights[t0:t0 + TPC]).reshape(NROWS, D)

        gidx = np.full((NSEG, 128, BPT * IDXCOLS), -1, np.int16)
        sidx = np.full((NSEG, 128, BPT * IDXCOLS), -1, np.int16)
        cnt = np.zeros(NSEG * BPT, np.int32)
        for s in range(NSEG):
            t = s // cfg.SPT
            so = (s % cfg.SPT) * cfg.SEGSZ
            seg = indices[t0 + t, so:so + cfg.SEGSZ].astype(np.int64)
            blk = seg // cfg.BLKSZ
            off = seg % cfg.BLKSZ
            # sort by (block, row-offset): ascending row order within each
            # bucket turns the gather's random 256B HBM reads into a
            # near-sequential sweep of its 8MB block (~1.5KB mean stride)
            order = np.lexsort((off, blk))
            counts = np.bincount(blk, minlength=BPT)
            if counts.max() > cfg.NPAD or counts.min() < 1:
                return None
            starts = np.zeros(BPT + 1, np.int64)
            np.cumsum(counts, out=starts[1:])
            for b in range(BPT):
                sel = order[starts[b]:starts[b + 1]]
                n = len(sel)
                gbuf = np.full(cfg.NPAD, -1, np.int16)
                gbuf[:n] = off[sel].astype(np.int16)
                pbuf = np.full(cfg.NPAD, -1, np.int16)
                pbuf[:n] = sel.astype(np.int16)
                gidx[s, :, b * IDXCOLS:(b + 1) * IDXCOLS] = np.tile(
                    gbuf.reshape(IDXCOLS, 16).T, (8, 1))
                sidx[s, :, b * IDXCOLS:(b + 1) * IDXCOLS] = np.tile(
                    pbuf.reshape(IDXCOLS, 16).T, (8, 1))
                cnt[s * BPT + b] = n
        in_maps.append({
            "w": w_c,
            "gidx": gidx.reshape(NSEG * 128, BPT * IDXCOLS),
            "sidx": sidx.reshape(NSEG * 128, BPT * IDXCOLS),
            "cnt": cnt.reshape(1, NSEG * BPT),
        })
    return in_maps


def unshard(outs: dict) -> np.ndarray:
    o = np.concatenate(list(outs["out"]), axis=0)
    if o.dtype != np.float32:
        o = o[:, :D].astype(np.float32)
    return np.ascontiguousarray(o)


# ------------------------------------------------- v1 fallback (generic)

P = 128
M = N // P
K1 = 128
NBUF = 4


def build_nc_v1(repeat=1):
    key = ("v1", repeat)
    if key in _NC_CACHE:
        return _NC_CACHE[key]
    NCHUNKS = M // K1
    nc = bass.Bass("TRN2", target_bir_lowering=False, debug=False)
    idx = nc.dram_tensor("idx", [P, M], mybir.dt.int32, kind="ExternalInput")
    w = nc.dram_tensor("w", [NROWS, D], mybir.dt.float32,
                       kind="ExternalInput")
    out = nc.dram_tensor("out", [N, D], mybir.dt.float32,
                         kind="ExternalOutput")
    out_v = out.ap().rearrange("(p m) d -> p (m d)", p=P)

    with ExitStack() as ctx:
        idx_tile = ctx.enter_context(
            nc.sbuf_tensor("idx_tile", [P, M], mybir.dt.int32))
        dst = ctx.enter_context(
            nc.sbuf_tensor("dst", [P, NBUF * K1 * D], mybir.dt.float32))
        idx_sem = ctx.enter_context(nc.semaphore("idx_sem"))
        g_sems = [ctx.enter_context(nc.semaphore(f"g_sem{s}"))
                  for s in range(NBUF)]
        s_sems = [ctx.enter_context(nc.semaphore(f"s_sem{s}"))
                  for s in range(NBUF)]
        block = ctx.enter_context(nc.Block())

        @block.gpsimd
        def _(gpsimd):
            gpsimd.dma_start(idx_tile[:], idx.ap()).then_inc(idx_sem, 16)
            gpsimd.wait_ge(idx_sem, 16)
            for c in range(NCHUNKS * repeat):
                cc = c % NCHUNKS
                s, k = c % NBUF, c // NBUF
                if k > 0:
                    gpsimd.wait_ge(s_sems[s], 16 * k)
                base = s * K1 * D
                for r in range(K1):
                    gpsimd.indirect_dma_start(
                        out=dst[:, base + r * D:base + (r + 1) * D],
                        out_offset=None,
                        in_=w.ap(),
                        in_offset=bass.IndirectOffsetOnAxis(
                            ap=idx_tile[:, cc * K1 + r:cc * K1 + r + 1], axis=0
                        ),
                    ).then_inc(g_sems[s], 16)

        @block.sync
        def _(sync):
            for c in range(NCHUNKS * repeat):
                cc = c % NCHUNKS
                s, k = c % NBUF, c // NBUF
                sync.wait_ge(g_sems[s], 16 * K1 * (k + 1))
                sync.dma_start(
                    out_v[:, cc * K1 * D:(cc + 1) * K1 * D],
                    dst[:, s * K1 * D:(s + 1) * K1 * D],
                ).then_inc(s_sems[s], 16)

    _NC_CACHE[key] = nc
    return nc


def _shard_inputs_v1(indices, weights):
    in_maps = []
    for c in range(NCORES):
        t0 = c * TPC
        w_c = np.ascontiguousarray(weights[t0:t0 + TPC]).reshape(NROWS, D)
        idx_c = indices[t0:t0 + TPC].astype(np.int64, copy=True)
        idx_c += (np.arange(TPC, dtype=np.int64) * R)[:, None]
        idx_c = idx_c.astype(np.int32).reshape(P, M)
        in_maps.append({"idx": idx_c, "w": w_c})
    return in_maps


# ------------------------------------------------------------- entrypoint

def kernel(indices: np.ndarray, weights: np.ndarray, **run_kwargs) -> np.ndarray:
    indices = np.asarray(indices, dtype=np.int32)
    weights = np.asarray(weights, dtype=np.float32)
    assert indices.shape == (T, L) and weights.shape == (T, R, D)

    in_maps = shard_inputs(indices, weights)
    if in_maps is not None:
        nc = build_nc()
    else:  # pathological index distribution: generic indirect-DMA path
        nc = build_nc_v1()
        in_maps = _shard_inputs_v1(indices, weights)

    res = run_bass_kernel_spmd(nc, in_maps, core_ids=list(range(NCORES)),
                               **run_kwargs)
    out = np.concatenate([r["out"] for r in res.results], axis=0)
    if out.dtype != np.float32:
        out = out[:, :D].astype(np.float32)
    kernel.last_results = res
    return np.ascontiguousarray(out)


# revision 8
# speedup vs baseline: 1.9484x; 1.9484x over previous
"""GroupedEmbedding lookup on 8 Trainium2 NeuronCores.

Sharding: table-wise (torchrec-style), 2 tables per core. Each core
holds its own [2*R, D] weight slab and that slab's index slices; its
output is a contiguous [2*L, D] block of the final [T*L, D] output, so
the un-shard is a plain concatenation (no device all-to-all needed).

Device algorithm (per core), v2 "MoE-style" path:
  One dma_gather + one dma_scatter_add extended instruction per
  (output-segment, weight-block) bucket. The int16 index limit of these
  instructions forces two-level blocking:
   - output positions are processed in NSEG=8 segments of SEGSZ=32768
     consecutive lookups (scatter offsets are segment-relative int16);
     each segment's lookups belong to exactly one table (SEGSZ | L);
   - each table's rows are split into BPT=7 blocks of BLKSZ=32768 rows
     (gather offsets are block-relative int16).
  Host pre-buckets each segment's lookups by weight block (stable
  counting sort). Per bucket: dma_gather pulls the bucket's 256B rows
  into SBUF (token i -> partition i%128, col i//128), dma_scatter_add
  writes them to their original positions in the output segment (the
  PJRT path zero-initializes outputs via donated buffers, each position
  is written exactly once, so += is =). Bucket counts are runtime
  register loads, so one compiled program serves any input data; idx
  buffers are padded to NPAD with trailing -1 (the documented "ignored"
  terminator) and replicated across the 8 gpsimd-core 16-partition
  windows as the ucode requires. single_packet=False is required:
  single-packet mode breaks above 1024 descriptors (64-descriptor HW
  packet ceiling x 16 SDMA engines).

  Two host-side orderings matter: buckets are sorted by ascending row
  offset so each gather's 256B HBM reads sweep its 8MB block nearly
  sequentially (29% faster than random order), and each bucket is split
  into even/odd interleaved chunks issued on separate SWDGE queue pairs
  (gathers q0/q2, scatters q1/q3) — the two chunk streams walk the same
  sorted address sequence in lockstep, so per-engine outstanding reads
  double without losing HBM row-buffer locality (a further 1.48x).
  Bucket counts ride in registers so the compiled program is input-
  independent. Measured per-iteration device time (8 cores, slope
  timing): ~1.69ms vs ~3.15ms for the indirect-DMA fallback.

Fallback path (bucket overflow/empty, i.e. severely non-uniform index
distributions): the original indirect_dma_start kernel, 128 rows per
instruction — slower but fully general.
"""
from contextlib import ExitStack
from dataclasses import dataclass
from functools import cached_property

import numpy as np

import concourse.bass as bass
import concourse.bacc as bacc
import concourse.mybir as mybir
from concourse.bass_utils import run_bass_kernel_spmd

T = 16          # tables
R = 200000      # rows per table
D = 64          # embedding dim (64 f32 = 256B rows)
L = 131072      # lookups per table
NCORES = 8
TPC = T // NCORES
NROWS = TPC * R     # 400000 rows per core
N = TPC * L         # 262144 lookups per core


# ---------------------------------------------------------------- v2 path

@dataclass(frozen=True)
class Cfg:
    SEGSZ: int = 32768   # output segment (int16 position limit)
    BLKSZ: int = 32768   # weight block (int16 row-offset limit)
    NPAD: int = 5888     # bucket capacity, mult of 256 (mean 5369 + 10s)
    NSLOT: int = 6       # data-tile pipeline slots
    LOOKAHEAD: int = 3   # chunk-gathers in flight before first scatter
    NQ: int = 4          # SWDGE queues
    CH: int = 2          # even/odd interleaved chunks per bucket
    HALF_OUT: bool = False  # cast to bf16 on-chip, 128B scatter descriptors

    @cached_property
    def SPT(self):
        return L // self.SEGSZ

    @cached_property
    def NSEG(self):
        return TPC * self.SPT

    @cached_property
    def BPT(self):
        return -(-R // self.BLKSZ)

    @cached_property
    def IDXCOLS(self):
        return self.NPAD // 16

    @cached_property
    def CPAD(self):
        return self.NPAD // self.CH

    @cached_property
    def CC(self):
        return self.CPAD // 16


CFG = Cfg()
_NC_CACHE = {}


def build_nc(cfg: Cfg = CFG, repeat=1):
    if (cfg, repeat) in _NC_CACHE:
        return _NC_CACHE[(cfg, repeat)]
    NSEG, BPT, IDXCOLS = cfg.NSEG, cfg.BPT, cfg.IDXCOLS
    NPAD, NSLOT, LOOKAHEAD = cfg.NPAD, cfg.NSLOT, cfg.LOOKAHEAD
    SEGSZ, BLKSZ, SPT = cfg.SEGSZ, cfg.BLKSZ, cfg.SPT
    CH, CPAD, CC = cfg.CH, cfg.CPAD, cfg.CC
    NREG = LOOKAHEAD + 2
    assert CPAD % 128 == 0 and L % SEGSZ == 0 and NSEG % 2 == 0
    assert NSLOT >= LOOKAHEAD + 2

    # Bacc: its compile() handles the DMAGatherAnt/ScatterAddAnt extended
    # instructions (insert_library_loads + InstISA codegen); raw Bass +
    # walrus codegen rejects them ("ISA wrong length").
    nc = bacc.Bacc("TRN2", debug=False, num_swdge_queues=cfg.NQ)

    def gq(k):
        return (0, 0, 0, k % 2 * 2)[cfg.NQ - 1]

    def sq(k):
        return (0, 1, 1, k % 2 * 2 + 1)[cfg.NQ - 1]

    w = nc.dram_tensor("w", [NROWS, D], mybir.dt.float32,
                       kind="ExternalInput")
    gidx = nc.dram_tensor("gidx", [NSEG * 128, BPT * IDXCOLS],
                          mybir.dt.int16, kind="ExternalInput")
    sidx = nc.dram_tensor("sidx", [NSEG * 128, BPT * IDXCOLS],
                          mybir.dt.int16, kind="ExternalInput")
    cnt = nc.dram_tensor("cnt", [1, NSEG * BPT * CH], mybir.dt.int32,
                         kind="ExternalInput")
    if cfg.HALF_OUT:
        # bf16 rows padded to 256B (scatter stride must divide by 256B)
        out = nc.dram_tensor("out", [N, 2 * D], mybir.dt.bfloat16,
                             kind="ExternalOutput")
    else:
        out = nc.dram_tensor("out", [N, D], mybir.dt.float32,
                             kind="ExternalOutput")

    with ExitStack() as ctx:
        gtile = [ctx.enter_context(
            nc.sbuf_tensor(f"gtile{p}", [128, BPT * IDXCOLS], mybir.dt.int16))
            for p in range(2)]
        stile = [ctx.enter_context(
            nc.sbuf_tensor(f"stile{p}", [128, BPT * IDXCOLS], mybir.dt.int16))
            for p in range(2)]
        ctile = ctx.enter_context(
            nc.sbuf_tensor("ctile", [1, NSEG * BPT * CH], mybir.dt.int32))
        data = [ctx.enter_context(
            nc.sbuf_tensor(f"data{i}", [128, (CPAD // 128) * D],
                           mybir.dt.float32))
            for i in range(NSLOT)]
        data3 = [d.ap().rearrange("p (a b) -> p a b", b=D) for d in data]
        if cfg.HALF_OUT:
            half = [ctx.enter_context(
                nc.sbuf_tensor(f"half{i}", [128, (NPAD // 128) * D],
                               mybir.dt.bfloat16))
                for i in range(NSLOT)]
            half3 = [h.ap().rearrange("p (a b) -> p a b", b=D) for h in half]
            c_sems = [ctx.enter_context(nc.semaphore(f"c_sem{i}"))
                      for i in range(NSLOT)]

        cnt_sem = ctx.enter_context(nc.semaphore("cnt_sem"))
        idx_sems = [ctx.enter_context(nc.semaphore(f"idx_sem{p}"))
                    for p in range(2)]
        g_sems = [ctx.enter_context(nc.semaphore(f"g_sem{i}"))
                  for i in range(NSLOT)]
        s_sems = [ctx.enter_context(nc.semaphore(f"s_sem{i}"))
                  for i in range(NSLOT)]
        block = ctx.enter_context(nc.Block())

        @block.sync
        def _(sync):
            sync.dma_start(ctile[:], cnt.ap()).then_inc(cnt_sem, 16)
            for gs in range(NSEG * repeat):
                s = gs % NSEG
                p = gs % 2
                if gs >= 2:
                    # idx tiles of parity p were last read (at desc-gen
                    # time) by segment gs-2; wait for each of its bucket
                    # DMAs (completion implies the desc-gen read is done).
                    for kk in range((gs - 2) * BPT * CH,
                                    (gs - 1) * BPT * CH):
                        sync.wait_ge(g_sems[kk % NSLOT],
                                     16 * (kk // NSLOT + 1))
                        sync.wait_ge(s_sems[kk % NSLOT],
                                     16 * (kk // NSLOT + 1))
                sync.dma_start(gtile[p][:], gidx.ap()[s * 128:(s + 1) * 128, :]
                               ).then_inc(idx_sems[p], 16)
                sync.dma_start(stile[p][:], sidx.ap()[s * 128:(s + 1) * 128, :]
                               ).then_inc(idx_sems[p], 16)

        if cfg.HALF_OUT:
            # fp32 -> bf16 cast between gather and scatter, split across
            # the DVE and ACT engines (both otherwise idle).
            def cast_block(eng, parity):
                for k in range(NSEG * repeat * BPT):
                    if k % 2 != parity:
                        continue
                    slot, rnd = k % NSLOT, k // NSLOT
                    eng.wait_ge(g_sems[slot], 16 * (rnd + 1))
                    if rnd > 0:
                        eng.wait_ge(s_sems[slot], 16 * rnd)
                    if parity == 0:
                        inst = eng.tensor_copy(half[slot][:], data[slot][:])
                    else:
                        inst = eng.copy(half[slot][:], data[slot][:])
                    inst.then_inc(c_sems[slot], 1)

            @block.vector
            def _(vector):
                cast_block(vector, 0)

            @block.scalar
            def _(scalar):
                cast_block(scalar, 1)

        @block.gpsimd
        def _(gpsimd):
            regs = [gpsimd.alloc_register(f"cnt_reg{i}") for i in range(NREG)]
            gpsimd.wait_ge(cnt_sem, 16)
            pend = []

            def issue_scatter(k):
                s = (k // (BPT * CH)) % NSEG
                b = (k % (BPT * CH)) // CH
                ch = k % CH
                slot, rnd = k % NSLOT, k // NSLOT
                sidx_ap = stile[(k // (BPT * CH)) % 2][
                    :, (b * CH + ch) * CC:(b * CH + ch + 1) * CC]
                if cfg.HALF_OUT:
                    gpsimd.wait_ge(c_sems[slot], rnd + 1)
                    gpsimd.dma_scatter_add(
                        out.ap()[s * SEGSZ:(s + 1) * SEGSZ, 0:D],
                        half3[slot], sidx_ap,
                        NPAD, regs[k % NREG], D, elem_step=2 * D,
                        queue_num=sq(k), single_packet=False,
                    ).then_inc(s_sems[slot], 16)
                    return
                gpsimd.wait_ge(g_sems[slot], 16 * (rnd + 1))
                gpsimd.dma_scatter_add(
                    out.ap()[s * SEGSZ:(s + 1) * SEGSZ, :],
                    data3[slot], sidx_ap,
                    CPAD, regs[k % NREG], D, elem_step=D,
                    queue_num=sq(k), single_packet=False,
                ).then_inc(s_sems[slot], 16)

            for gs in range(NSEG * repeat):
                s = gs % NSEG
                t = s // SPT
                gpsimd.wait_ge(idx_sems[gs % 2], 32 * (gs // 2 + 1))
                for b in range(BPT):
                    for ch in range(CH):
                        k = (gs * BPT + b) * CH + ch
                        slot, rnd = k % NSLOT, k // NSLOT
                        if rnd > 0:
                            gpsimd.wait_ge(s_sems[slot], 16 * rnd)
                        ci = (s * BPT + b) * CH + ch
                        gpsimd.reg_load(regs[k % NREG], ctile[:, ci:ci + 1])
                        r0 = t * R + b * BLKSZ
                        r1 = t * R + min(R, (b + 1) * BLKSZ)
                        gpsimd.dma_gather(
                            data3[slot],
                            w.ap()[r0:r1, :],
                            gtile[gs % 2][
                                :, (b * CH + ch) * CC:(b * CH + ch + 1) * CC],
                            CPAD, regs[k % NREG], D, elem_step=D,
                            queue_num=gq(k), single_packet=False,
                        ).then_inc(g_sems[slot], 16)
                        pend.append(k)
                        if len(pend) > LOOKAHEAD:
                            issue_scatter(pend.pop(0))
            for k in pend:
                issue_scatter(k)

    nc.compile()
    _NC_CACHE[(cfg, repeat)] = nc
    return nc


def shard_inputs(indices: np.ndarray, weights: np.ndarray, cfg: Cfg = CFG):
    """Full inputs -> per-core in_maps (bucketed, wrapped, replicated).

    Returns None when a bucket overflows NPAD or a feasible bucket is
    empty (won't happen for uniform indices) -> caller falls back to the
    generic v1 path.
    """
    NSEG, BPT, IDXCOLS = cfg.NSEG, cfg.BPT, cfg.IDXCOLS
    in_maps = []
    for c in range(NCORES):
        t0 = c * TPC
        w_c = np.ascontiguousarray(weights[t0:t0 + TPC]).reshape(NROWS, D)

        gidx = np.full((NSEG, 128, BPT * IDXCOLS), -1, np.int16)
        sidx = np.full((NSEG, 128, BPT * IDXCOLS), -1, np.int16)
        cnt = np.zeros(NSEG * BPT * cfg.CH, np.int32)
        for s in range(NSEG):
            t = s // cfg.SPT
            so = (s % cfg.SPT) * cfg.SEGSZ
            seg = indices[t0 + t, so:so + cfg.SEGSZ].astype(np.int64)
            blk = seg // cfg.BLKSZ
            off = seg % cfg.BLKSZ
            # sort by (block, row-offset): ascending row order within each
            # bucket turns the gather's random 256B HBM reads into a
            # near-sequential sweep of its 8MB block (~1.5KB mean stride)
            order = np.lexsort((off, blk))
            counts = np.bincount(blk, minlength=BPT)
            if counts.max() > cfg.NPAD or counts.min() < cfg.CH:
                return None
            starts = np.zeros(BPT + 1, np.int64)
            np.cumsum(counts, out=starts[1:])
            for b in range(BPT):
                sel = order[starts[b]:starts[b + 1]]
                # even/odd interleave: both chunks sweep the same sorted
                # address sequence in lockstep on separate SWDGE queues
                for ch in range(cfg.CH):
                    sub = sel[ch::cfg.CH]
                    n = len(sub)
                    gbuf = np.full(cfg.CPAD, -1, np.int16)
                    gbuf[:n] = off[sub].astype(np.int16)
                    pbuf = np.full(cfg.CPAD, -1, np.int16)
                    pbuf[:n] = sub.astype(np.int16)
                    col = (b * cfg.CH + ch) * cfg.CC
                    gidx[s, :, col:col + cfg.CC] = np.tile(
                        gbuf.reshape(cfg.CC, 16).T, (8, 1))
                    sidx[s, :, col:col + cfg.CC] = np.tile(
                        pbuf.reshape(cfg.CC, 16).T, (8, 1))
                    cnt[(s * BPT + b) * cfg.CH + ch] = n
        in_maps.append({
            "w": w_c,
            "gidx": gidx.reshape(NSEG * 128, BPT * IDXCOLS),
            "sidx": sidx.reshape(NSEG * 128, BPT * IDXCOLS),
            "cnt": cnt.reshape(1, NSEG * BPT * cfg.CH),
        })
    return in_maps


def unshard(outs: dict) -> np.ndarray:
    o = np.concatenate(list(outs["out"]), axis=0)
    if o.dtype != np.float32:
        o = o[:, :D].astype(np.float32)
    return np.ascontiguousarray(o)


# ------------------------------------------------- v1 fallback (generic)

P = 128
M = N // P
K1 = 128
NBUF = 4


def build_nc_v1(repeat=1):
    key = ("v1", repeat)
    if key in _NC_CACHE:
        return _NC_CACHE[key]
    NCHUNKS = M // K1
    nc = bass.Bass("TRN2", target_bir_lowering=False, debug=False)
    idx = nc.dram_tensor("idx", [P, M], mybir.dt.int32, kind="ExternalInput")
    w = nc.dram_tensor("w", [NROWS, D], mybir.dt.float32,
                       kind="ExternalInput")
    out = nc.dram_tensor("out", [N, D], mybir.dt.float32,
                         kind="ExternalOutput")
    out_v = out.ap().rearrange("(p m) d -> p (m d)", p=P)

    with ExitStack() as ctx:
        idx_tile = ctx.enter_context(
            nc.sbuf_tensor("idx_tile", [P, M], mybir.dt.int32))
        dst = ctx.enter_context(
            nc.sbuf_tensor("dst", [P, NBUF * K1 * D], mybir.dt.float32))
        idx_sem = ctx.enter_context(nc.semaphore("idx_sem"))
        g_sems = [ctx.enter_context(nc.semaphore(f"g_sem{s}"))
                  for s in range(NBUF)]
        s_sems = [ctx.enter_context(nc.semaphore(f"s_sem{s}"))
                  for s in range(NBUF)]
        block = ctx.enter_context(nc.Block())

        @block.gpsimd
        def _(gpsimd):
            gpsimd.dma_start(idx_tile[:], idx.ap()).then_inc(idx_sem, 16)
            gpsimd.wait_ge(idx_sem, 16)
            for c in range(NCHUNKS * repeat):
                cc = c % NCHUNKS
                s, k = c % NBUF, c // NBUF
                if k > 0:
                    gpsimd.wait_ge(s_sems[s], 16 * k)
                base = s * K1 * D
                for r in range(K1):
                    gpsimd.indirect_dma_start(
                        out=dst[:, base + r * D:base + (r + 1) * D],
                        out_offset=None,
                        in_=w.ap(),
                        in_offset=bass.IndirectOffsetOnAxis(
                            ap=idx_tile[:, cc * K1 + r:cc * K1 + r + 1], axis=0
                        ),
                    ).then_inc(g_sems[s], 16)

        @block.sync
        def _(sync):
            for c in range(NCHUNKS * repeat):
                cc = c % NCHUNKS
                s, k = c % NBUF, c // NBUF
                sync.wait_ge(g_sems[s], 16 * K1 * (k + 1))
                sync.dma_start(
                    out_v[:, cc * K1 * D:(cc + 1) * K1 * D],
                    dst[:, s * K1 * D:(s + 1) * K1 * D],
                ).then_inc(s_sems[s], 16)

    _NC_CACHE[key] = nc
    return nc


def _shard_inputs_v1(indices, weights):
    in_maps = []
    for c in range(NCORES):
        t0 = c * TPC
        w_c = np.ascontiguousarray(weights[t0:t0 + TPC]).reshape(NROWS, D)
        idx_c = indices[t0:t0 + TPC].astype(np.int64, copy=True)
        idx_c += (np.arange(TPC, dtype=np.int64) * R)[:, None]
        idx_c = idx_c.astype(np.int32).reshape(P, M)
        in_maps.append({"idx": idx_c, "w": w_c})
    return in_maps


# ------------------------------------------------------------- entrypoint

def kernel(indices: np.ndarray, weights: np.ndarray, **run_kwargs) -> np.ndarray:
    indices = np.asarray(indices, dtype=np.int32)
    weights = np.asarray(weights, dtype=np.float32)
    assert indices.shape == (T, L) and weights.shape == (T, R, D)

    in_maps = shard_inputs(indices, weights)
    if in_maps is not None:
        nc = build_nc()
    else:  # pathological index distribution: generic indirect-DMA path
        nc = build_nc_v1()
        in_maps = _shard_inputs_v1(indices, weights)

    res = run_bass_kernel_spmd(nc, in_maps, core_ids=list(range(NCORES)),
                               **run_kwargs)
    out = np.concatenate([r["out"] for r in res.results], axis=0)
    if out.dtype != np.float32:
        out = out[:, :D].astype(np.float32)
    kernel.last_results = res
    return np.ascontiguousarray(out)
